# revision 6
# baseline (speedup 1.0000x reference)
"""Multi-head self-attention (N=2, S=4096, D=1024, H=16) on 8 trn2 cores.

Sharding: data-parallel over batch (2) x tensor-parallel over head groups
(4 heads per core). Core c handles batch b=c//4, head group g=c%4
(heads 4g..4g+3, i.e. output columns 256g..256g+256). No cross-device
comms: heads are independent.

Per-core device kernel (all matmuls in float32r: full PE rate, ~1e-4 rel):
  1. Projections: qT,kT [256,4096] and v [4096,256] from xT [1024,4096]
     (host passes x/W pre-transposed; pure layout prep).
     v is stored interleaved with a ones column per head ("vaug",
     [4096, 4*65]) so the PV matmul also produces softmax denominators.
  2. Attention per head, flash-style over the 4096x4096 score matrix:
     ST chunk [j=128, i=1024] = kT_h.T @ qT_h  (PE, head pairs row-tiled)
     E = exp(ST/8)                             (ScalarE, PSUM -> SBUF)
     OT[65, i] += vaug_j.T @ E                 (PE, accumulated in PSUM)
     rows 0..63 of OT = head output (d x i), row 64 = sum_j E = denom.
  3. Epilogue per 128-query block: PE-transpose OT -> [i, 65], DVE
     reciprocal+scale by denom, DMA out.
"""

import numpy as np

import concourse.bacc as bacc
import concourse.tile as tile
import concourse.mybir as mybir
from concourse.bass_utils import run_bass_kernel_spmd
from concourse.masks import make_identity

F32 = mybir.dt.float32
F32R = mybir.dt.float32r
Exp = mybir.ActivationFunctionType.Exp

N, S, D = 2, 4096, 1024
H = 16
HD = D // H                      # 64
N_CORES = 8
HPC = H // (N_CORES // N)        # heads per core = 4
MPC = HPC * HD                   # out columns per core = 256
SCALE = 1.0 / np.sqrt(HD)        # post-matmul softmax scale

IC = 1024                        # i-chunk (query cols per exp instruction)
N_IC = S // IC                   # 4
N_JC = S // 128                  # 32 key chunks
N_SC = S // 512                  # 8 projection s-chunks
N_DT = D // 128                  # 8 contraction tiles


def build_attention_kernel():
    nc = bacc.Bacc(
        "TRN2", target_bir_lowering=False, debug=False,
        enable_asserts=False, num_devices=N_CORES,
    )
    xT = nc.dram_tensor("xT", [D, S], F32R, kind="ExternalInput").ap()
    wqT = nc.dram_tensor("wqT", [D, MPC], F32R, kind="ExternalInput").ap()
    wkT = nc.dram_tensor("wkT", [D, MPC], F32R, kind="ExternalInput").ap()
    wvT = nc.dram_tensor("wvT", [D, MPC], F32R, kind="ExternalInput").ap()
    out = nc.dram_tensor("out", [S, MPC], F32, kind="ExternalOutput").ap()

    with tile.TileContext(nc) as tc:
        _emit(tc, xT, wqT, wkT, wvT, out)
    nc.compile()
    return nc


def _emit(tc, xT, wqT, wkT, wvT, out):
    nc = tc.nc
    with tc.tile_pool(name="persist", bufs=1) as persist:
        # persistent SBUF tensors
        w_sb = {}
        for name, w in (("q", wqT), ("k", wkT), ("v", wvT)):
            t = persist.tile([128, N_DT, MPC], F32R, tag=f"w{name}")
            for dt in range(N_DT):
                nc.sync.dma_start(out=t[:, dt, :], in_=w[dt * 128:(dt + 1) * 128, :])
            w_sb[name] = t
        qT_sb = persist.tile([128, 2, S], F32R, tag="qT")   # [m 2x128, s]
        kT_sb = persist.tile([128, 2, S], F32R, tag="kT")
        vaug = persist.tile([128, N_JC, HPC * (HD + 1)], F32R, tag="vaug")
        ident = persist.tile([128, 128], F32, tag="ident")
        make_identity(nc, ident)
        # ones columns of vaug (col 64 of each head's 65-wide strip);
        # memset on an fp32r tile fails the ISA check, so stage fp32 ones
        # and DVE-copy them (copy output counts as "rounded to fp32r").
        ones_src = persist.tile([128, HPC], F32, tag="ones")
        nc.vector.memset(ones_src, 1.0)
        for jc in range(N_JC):
            nc.vector.tensor_copy(
                vaug[:, jc, :].rearrange(
                    "p (h c) -> p h c", c=HD + 1)[:, :, HD:HD + 1],
                ones_src[:].rearrange("p (h c) -> p h c", c=1),
            )

        # ---- phase 1: projections ----
        with (
            tc.tile_pool(name="xload", bufs=2) as xload,
            tc.tile_pool(name="ppsum", bufs=4, space="PSUM") as ppsum,
        ):
            for sc in range(N_SC):
                s0 = sc * 512
                x_t = xload.tile([128, N_DT, 512], F32R, tag="x")
                for dt in range(N_DT):
                    nc.sync.dma_start(
                        out=x_t[:, dt, :],
                        in_=xT[dt * 128:(dt + 1) * 128, s0:s0 + 512],
                    )
                # qT, kT: psum [128 m, 512 s] per m-tile
                for name, dst in (("q", qT_sb), ("k", kT_sb)):
                    for mt in range(2):
                        ps = ppsum.tile([128, 512], F32, tag="pqk")
                        for dt in range(N_DT):
                            nc.tensor.matmul(
                                ps[:],
                                w_sb[name][:, dt, mt * 128:(mt + 1) * 128],
                                x_t[:, dt, :],
                                start=(dt == 0), stop=(dt == N_DT - 1),
                            )
                        nc.vector.tensor_copy(dst[:, mt, s0:s0 + 512], ps[:])
                # v: psum [128 s, 256 m] per s-subtile, scattered into vaug
                for st in range(4):
                    ps = ppsum.tile([128, MPC], F32, tag="pv")
                    for dt in range(N_DT):
                        nc.tensor.matmul(
                            ps[:],
                            x_t[:, dt, st * 128:(st + 1) * 128],
                            w_sb["v"][:, dt, :],
                            start=(dt == 0), stop=(dt == N_DT - 1),
                        )
                    jc = sc * 4 + st
                    nc.vector.tensor_copy(
                        vaug[:, jc, :].rearrange(
                            "p (h c) -> p h c", c=HD + 1)[:, :, 0:HD],
                        ps[:].rearrange("p (h d) -> p h d", d=HD),
                    )

        # ---- phase 2: attention ----
        # PSUM budget (8 banks): st 2 bufs x 2 banks + ot 2 + tr 2 = 8
        with (
            tc.tile_pool(name="stp", bufs=2, space="PSUM") as stp,
            tc.tile_pool(name="otp", bufs=1, space="PSUM") as otp,
            tc.tile_pool(name="trp", bufs=2, space="PSUM") as trp,
            tc.tile_pool(name="esb", bufs=3) as esb,
            tc.tile_pool(name="episb", bufs=2) as episb,
            tc.tile_pool(name="osb", bufs=3) as osb,
        ):
            for h in range(HPC):
                p0 = (h % 2) * HD            # partition base within kT/qT tile
                mt = h // 2                  # m-tile index
                for ic in range(N_IC):
                    i0 = ic * IC
                    ot_ps = otp.tile([HD + 1, IC], F32, tag="ot")
                    for jc in range(N_JC):
                        j0 = jc * 128
                        st_ps = stp.tile([128, IC], F32, tag="st")
                        for half in range(IC // 512):
                            c0 = half * 512
                            nc.tensor.matmul(
                                st_ps[:, c0:c0 + 512],
                                kT_sb[p0:p0 + HD, mt, j0:j0 + 128],
                                qT_sb[p0:p0 + HD, mt, i0 + c0:i0 + c0 + 512],
                                start=True, stop=True,
                            )
                        e_t = esb.tile([128, IC], F32R, tag="e")
                        nc.scalar.activation(
                            e_t[:], st_ps[:], Exp, bias=0.0, scale=SCALE,
                        )
                        lhsT_v = vaug[:, jc, h * (HD + 1):(h + 1) * (HD + 1)]
                        for half in range(IC // 512):
                            c0 = half * 512
                            nc.tensor.matmul(
                                ot_ps[:, c0:c0 + 512],
                                lhsT_v,
                                e_t[:, c0:c0 + 512],
                                start=(jc == 0), stop=(jc == N_JC - 1),
                            )
                    # epilogue: normalize + transpose + store
                    ot_sb = episb.tile([HD + 1, IC], F32, tag="eo")
                    nc.vector.tensor_copy(ot_sb[:], ot_ps[:])
                    for bi in range(IC // 128):
                        tr = trp.tile([128, HD + 1], F32, tag="tr")
                        nc.tensor.transpose(
                            tr[:],
                            ot_sb[:, bi * 128:(bi + 1) * 128],
                            ident[0:HD + 1, 0:HD + 1],
                        )
                        rec = osb.tile([128, 1], F32, tag="rec")
                        nc.vector.reciprocal(rec[:], tr[:, HD:HD + 1])
                        o_t = osb.tile([128, HD], F32, tag="o")
                        nc.vector.tensor_scalar_mul(o_t[:], tr[:, 0:HD], rec[:])
                        r0 = i0 + bi * 128
                        nc.sync.dma_start(
                            out=out[r0:r0 + 128, h * HD:(h + 1) * HD],
                            in_=o_t[:],
                        )


_NC_CACHE = None


def _get_nc():
    global _NC_CACHE
    if _NC_CACHE is None:
        _NC_CACHE = build_attention_kernel()
    return _NC_CACHE


def _build_in_maps(inputs):
    x = np.asarray(inputs["x"], dtype=np.float32)
    Wq = np.asarray(inputs["Wq"], dtype=np.float32)
    Wk = np.asarray(inputs["Wk"], dtype=np.float32)
    Wv = np.asarray(inputs["Wv"], dtype=np.float32)
    xTs = [np.ascontiguousarray(x[b].T) for b in range(N)]
    in_maps = []
    for c in range(N_CORES):
        b, g = divmod(c, N_CORES // N)
        rows = slice(g * MPC, (g + 1) * MPC)
        in_maps.append({
            "xT": xTs[b],
            "wqT": np.ascontiguousarray(Wq[rows].T),
            "wkT": np.ascontiguousarray(Wk[rows].T),
            "wvT": np.ascontiguousarray(Wv[rows].T),
        })
    return in_maps


def kernel(x, Wq, Wk, Wv):
    nc = _get_nc()
    in_maps = _build_in_maps({"x": x, "Wq": Wq, "Wk": Wk, "Wv": Wv})
    res = run_bass_kernel_spmd(nc, in_maps, core_ids=list(range(N_CORES)))

    full = np.empty((N, S, D), dtype=np.float32)
    for c in range(N_CORES):
        b, g = divmod(c, N_CORES // N)
        full[b, :, g * MPC:(g + 1) * MPC] = res.results[c]["out"]
    return full


if __name__ == "__main__":
    rng = np.random.default_rng(0)
    x = rng.standard_normal((N, S, D)).astype(np.float32)
    Wq = (rng.standard_normal((D, D)) / 32).astype(np.float32)
    Wk = (rng.standard_normal((D, D)) / 32).astype(np.float32)
    Wv = (rng.standard_normal((D, D)) / 32).astype(np.float32)
    got = kernel(x, Wq, Wk, Wv)
    print("kernel output:", got.shape, got.dtype)


# revision 8
# speedup vs baseline: 2.0496x; 2.0496x over previous
"""Multi-head self-attention (N=2, S=4096, D=1024, H=16) on 8 trn2 cores.

Sharding: data-parallel over batch (2) x tensor-parallel over head groups
(4 heads per core). Core c handles batch b=c//4, head group g=c%4
(heads 4g..4g+3, i.e. output columns 256g..256g+256). No cross-device
comms: heads are independent.

Per-core device kernel:
  1. Projections (fp32r matmuls: full PE rate, ~1e-4 rel):
     qT [256,4096], kTz [4 heads][128,4096] and v [4096,256] from
     xT [1024,4096] (host passes x/W pre-transposed; layout prep only).
     kTz stores each head's kT in its 64-row parity slot with the other
     64 rows ZEROED: the S matmul then runs with K=128 so the PE array
     is fully active and the HAM clock gate stays at full rate (half-
     array K=64 matmuls measure as "idle" and get throttled to 1.2GHz).
     v is stored in bf16, interleaved with a ones column per head
     ("vaug") so the PV matmul also produces softmax denominators, and
     padded so every PV weight load is a full 128 columns (FWL + full
     array activity; the extra output rows land in PSUM pad space).
  2. Attention per head, flash-style over the 4096x4096 score matrix:
     ST chunk [j=128, i=1024] = kTz_h.T @ qT  (PE, K=128)
     E = exp(ST/8) in bf16                    (ScalarE, PSUM -> SBUF)
     OT[128, i] += vaug_j.T @ E               (PE, rows 0..64 real)
     row 64 of OT = sum_j E = softmax denominator.
  3. Epilogue per 128-query block: PE-transpose OT -> [i, 65], DVE
     reciprocal+scale by denom, DMA out.
"""

import numpy as np

import concourse.bacc as bacc
import concourse.tile as tile
import concourse.mybir as mybir
from concourse.bass_utils import run_bass_kernel_spmd
from concourse.masks import make_identity

F32 = mybir.dt.float32
F32R = mybir.dt.float32r
BF16 = mybir.dt.bfloat16
Exp = mybir.ActivationFunctionType.Exp

N, S, D = 2, 4096, 1024
H = 16
HD = D // H                      # 64
N_CORES = 8
HPC = H // (N_CORES // N)        # heads per core = 4
MPC = HPC * HD                   # out columns per core = 256
SCALE = 1.0 / np.sqrt(HD)        # post-matmul softmax scale

IC = 1024                        # i-chunk (query cols per exp instruction)
N_IC = S // IC                   # 4
N_JC = S // 128                  # 32 key chunks
N_SC = S // 512                  # 8 projection s-chunks
N_DT = D // 128                  # 8 contraction tiles
VW = HD + 1                      # vaug stride per head (64 v + 1 ones)


def build_attention_kernel():
    nc = bacc.Bacc(
        "TRN2", target_bir_lowering=False, debug=False,
        enable_asserts=False, num_devices=N_CORES,
    )
    xT = nc.dram_tensor("xT", [D, S], F32R, kind="ExternalInput").ap()
    wqT = nc.dram_tensor("wqT", [D, MPC], F32R, kind="ExternalInput").ap()
    wkT = nc.dram_tensor("wkT", [D, MPC], F32R, kind="ExternalInput").ap()
    wvT = nc.dram_tensor("wvT", [D, MPC], F32R, kind="ExternalInput").ap()
    out = nc.dram_tensor("out", [S, MPC], F32, kind="ExternalOutput").ap()

    with tile.TileContext(nc) as tc:
        _emit(tc, xT, wqT, wkT, wvT, out)
    nc.compile()
    return nc


def _emit(tc, xT, wqT, wkT, wvT, out):
    nc = tc.nc
    with tc.tile_pool(name="persist", bufs=1) as persist:
        # persistent SBUF tensors
        w_sb = {}
        for name, w in (("q", wqT), ("k", wkT), ("v", wvT)):
            t = persist.tile([128, N_DT, MPC], F32R, tag=f"w{name}")
            for dt in range(N_DT):
                nc.sync.dma_start(out=t[:, dt, :], in_=w[dt * 128:(dt + 1) * 128, :])
            w_sb[name] = t
        qT_sb = persist.tile([128, 2, S], F32R, tag="qT")   # [m 2x128, s]
        # per-head kT planes, K=128 with off-parity rows zeroed
        kTz = persist.tile([128, HPC, S], F32R, tag="kTz")
        # bf16 v + ones, one 65-wide strip per head + 63 pad cols so the
        # PV lhsT can always be read 128 wide
        vaug = persist.tile([128, N_JC, HPC * VW + HD - 1], BF16, tag="vaug")
        ident = persist.tile([128, 128], F32, tag="ident")
        make_identity(nc, ident)

        ones_src = persist.tile([128, HPC], F32, tag="ones")
        nc.vector.memset(ones_src, 1.0)
        zero_src = persist.tile([128, 512], F32, tag="zeros")
        nc.vector.memset(zero_src, 0.0)
        # zero the off-parity halves of kTz (memset can't write fp32r)
        for h in range(HPC):
            z0 = 64 if h % 2 == 0 else 0
            for sc in range(N_SC):
                nc.vector.tensor_copy(
                    kTz[z0:z0 + 64, h, sc * 512:(sc + 1) * 512],
                    zero_src[z0:z0 + 64, :],
                )
        # vaug ones columns + zero pad columns
        for jc in range(N_JC):
            nc.vector.tensor_copy(
                vaug[:, jc, 0:HPC * VW].rearrange(
                    "p (h c) -> p h c", c=VW)[:, :, HD:HD + 1],
                ones_src[:].rearrange("p (h c) -> p h c", c=1),
            )
            nc.vector.memset(vaug[:, jc, HPC * VW:], 0.0)

        # ---- phase 1: projections ----
        with (
            tc.tile_pool(name="xload", bufs=2) as xload,
            tc.tile_pool(name="ppsum", bufs=4, space="PSUM") as ppsum,
        ):
            for sc in range(N_SC):
                s0 = sc * 512
                x_t = xload.tile([128, N_DT, 512], F32R, tag="x")
                for dt in range(N_DT):
                    nc.sync.dma_start(
                        out=x_t[:, dt, :],
                        in_=xT[dt * 128:(dt + 1) * 128, s0:s0 + 512],
                    )
                # qT / kTz: psum [128 m, 512 s] per m-tile
                for name in ("q", "k"):
                    for mt in range(2):
                        ps = ppsum.tile([128, 512], F32, tag="pqk")
                        for dt in range(N_DT):
                            nc.tensor.matmul(
                                ps[:],
                                w_sb[name][:, dt, mt * 128:(mt + 1) * 128],
                                x_t[:, dt, :],
                                start=(dt == 0), stop=(dt == N_DT - 1),
                            )
                        if name == "q":
                            nc.vector.tensor_copy(
                                qT_sb[:, mt, s0:s0 + 512], ps[:])
                        else:
                            # split the two heads into their kTz planes
                            for hh in range(2):
                                p0 = hh * HD
                                nc.vector.tensor_copy(
                                    kTz[p0:p0 + HD, mt * 2 + hh, s0:s0 + 512],
                                    ps[p0:p0 + HD, :],
                                )
                # v: psum [128 s, 256 m] per s-subtile -> vaug (bf16)
                for st in range(4):
                    ps = ppsum.tile([128, MPC], F32, tag="pv")
                    for dt in range(N_DT):
                        nc.tensor.matmul(
                            ps[:],
                            x_t[:, dt, st * 128:(st + 1) * 128],
                            w_sb["v"][:, dt, :],
                            start=(dt == 0), stop=(dt == N_DT - 1),
                        )
                    jc = sc * 4 + st
                    nc.vector.tensor_copy(
                        vaug[:, jc, 0:HPC * VW].rearrange(
                            "p (h c) -> p h c", c=VW)[:, :, 0:HD],
                        ps[:].rearrange("p (h d) -> p h d", d=HD),
                    )

        # ---- phase 2: attention ----
        # PSUM budget (8 banks): st 2 bufs x 2 banks + ot 2 + tr 2 = 8
        with (
            tc.tile_pool(name="stp", bufs=2, space="PSUM") as stp,
            tc.tile_pool(name="otp", bufs=1, space="PSUM") as otp,
            tc.tile_pool(name="trp", bufs=2, space="PSUM") as trp,
            tc.tile_pool(name="esb", bufs=3) as esb,
            tc.tile_pool(name="episb", bufs=2) as episb,
            tc.tile_pool(name="osb", bufs=3) as osb,
        ):
            for h in range(HPC):
                mt = h // 2                  # qT m-tile for this head
                for ic in range(N_IC):
                    i0 = ic * IC
                    ot_ps = otp.tile([128, IC], F32, tag="ot")
                    for jc in range(N_JC):
                        j0 = jc * 128
                        st_ps = stp.tile([128, IC], F32, tag="st")
                        for half in range(IC // 512):
                            c0 = half * 512
                            nc.tensor.matmul(
                                st_ps[:, c0:c0 + 512],
                                kTz[:, h, j0:j0 + 128],
                                qT_sb[:, mt, i0 + c0:i0 + c0 + 512],
                                start=True, stop=True,
                            )
                        e_t = esb.tile([128, IC], BF16, tag="e")
                        nc.scalar.activation(
                            e_t[:], st_ps[:], Exp, bias=0.0, scale=SCALE,
                        )
                        lhsT_v = vaug[:, jc, h * VW:h * VW + 128]
                        for half in range(IC // 512):
                            c0 = half * 512
                            nc.tensor.matmul(
                                ot_ps[:, c0:c0 + 512],
                                lhsT_v,
                                e_t[:, c0:c0 + 512],
                                start=(jc == 0), stop=(jc == N_JC - 1),
                            )
                    # epilogue: normalize + transpose + store
                    ot_sb = episb.tile([HD + 1, IC], F32, tag="eo")
                    nc.vector.tensor_copy(ot_sb[:], ot_ps[0:HD + 1, :])
                    for bi in range(IC // 128):
                        tr = trp.tile([128, HD + 1], F32, tag="tr")
                        nc.tensor.transpose(
                            tr[:],
                            ot_sb[:, bi * 128:(bi + 1) * 128],
                            ident[0:HD + 1, 0:HD + 1],
                        )
                        rec = osb.tile([128, 1], F32, tag="rec")
                        nc.vector.reciprocal(rec[:], tr[:, HD:HD + 1])
                        o_t = osb.tile([128, HD], F32, tag="o")
                        nc.vector.tensor_scalar_mul(o_t[:], tr[:, 0:HD], rec[:])
                        r0 = i0 + bi * 128
                        nc.sync.dma_start(
                            out=out[r0:r0 + 128, h * HD:(h + 1) * HD],
                            in_=o_t[:],
                        )


_NC_CACHE = None


def _get_nc():
    global _NC_CACHE
    if _NC_CACHE is None:
        _NC_CACHE = build_attention_kernel()
    return _NC_CACHE


def _build_in_maps(inputs):
    x = np.asarray(inputs["x"], dtype=np.float32)
    Wq = np.asarray(inputs["Wq"], dtype=np.float32)
    Wk = np.asarray(inputs["Wk"], dtype=np.float32)
    Wv = np.asarray(inputs["Wv"], dtype=np.float32)
    xTs = [np.ascontiguousarray(x[b].T) for b in range(N)]
    in_maps = []
    for c in range(N_CORES):
        b, g = divmod(c, N_CORES // N)
        rows = slice(g * MPC, (g + 1) * MPC)
        in_maps.append({
            "xT": xTs[b],
            "wqT": np.ascontiguousarray(Wq[rows].T),
            "wkT": np.ascontiguousarray(Wk[rows].T),
            "wvT": np.ascontiguousarray(Wv[rows].T),
        })
    return in_maps


def kernel(x, Wq, Wk, Wv):
    nc = _get_nc()
    in_maps = _build_in_maps({"x": x, "Wq": Wq, "Wk": Wk, "Wv": Wv})
    res = run_bass_kernel_spmd(nc, in_maps, core_ids=list(range(N_CORES)))

    full = np.empty((N, S, D), dtype=np.float32)
    for c in range(N_CORES):
        b, g = divmod(c, N_CORES // N)
        full[b, :, g * MPC:(g + 1) * MPC] = res.results[c]["out"]
    return full


if __name__ == "__main__":
    rng = np.random.default_rng(0)
    x = rng.standard_normal((N, S, D)).astype(np.float32)
    Wq = (rng.standard_normal((D, D)) / 32).astype(np.float32)
    Wk = (rng.standard_normal((D, D)) / 32).astype(np.float32)
    Wv = (rng.standard_normal((D, D)) / 32).astype(np.float32)
    got = kernel(x, Wq, Wk, Wv)
    print("kernel output:", got.shape, got.dtype)
